# revision 1
# baseline (speedup 1.0000x reference)
"""MSE-style custom loss on 8 Trainium2 NeuronCores — fp8 streaming.

reference: d = |input - target|; conditional 0.8 scale of d[0] when
d[0] in {3,4,5,6}; return mean(d*d).

Strategy (data-parallel, memory-bound; harness tolerance 2e-2):
  - Host: shard to 8 cores (4M elems each), cast a -> fp8_e4m3 and
    (-b) -> fp8_e4m3 (sign flip is exact).  Quantization alone gives
    ~7e-4 rel error on the final mean (measured offline), 25x under
    the gate, while cutting HBM traffic 4x vs fp32.
  - Device per core: stream fp8 tiles holding [a | -b] side by side.
    The tensor engine computes d = I.T @ a + I.T @ (-b) into PSUM in a
    single fp8 DoubleRow matmul per PSUM bank (0.5 cycles/row; weights
    [I | I] so the two k-tiles are a and -b).  PSUM groups
    of [128 x 2048] (4 banks) are then square-reduced: the scalar
    engine (Square activation + accum_out) takes most groups, the
    vector engine (tensor_tensor_reduce mult+add) takes the rest, so
    neither engine exceeds the DMA streaming time.
  - Host: sum partials in f64, apply the d[0] fixup, divide by N.
"""

import numpy as np
import ml_dtypes

N = 33554432
N_CORES = 8
SHARD = N // N_CORES          # 4194304
P = 128
FREE = SHARD // P             # 32768 fp8 bytes per partition
TILE_F = 8192                 # fp8 tile free size (8 KB/partition rows)
GROUP = 2048                  # psum group free size (4 banks of 512 fp32)
QUART = 512                   # one psum bank of fp32

_cache = {}


def _build(free):
    import concourse.tile as tile
    from concourse import bacc, mybir

    shard = P * free
    n_tiles = free // TILE_F if free >= TILE_F else 1
    tile_f = min(TILE_F, free)
    n_groups = free // GROUP
    assert free % GROUP == 0 and tile_f % GROUP == 0

    dve_groups = [g for g in range(n_groups) if g % 8 in (2, 4, 6)]
    act_groups = [g for g in range(n_groups) if g % 8 not in (2, 4, 6)]
    n_dve, n_act = len(dve_groups), len(act_groups)

    nc = bacc.Bacc("TRN2", target_bir_lowering=False, debug=False)
    a_d = nc.dram_tensor("input", [shard], mybir.dt.float8e4,
                         kind="ExternalInput").ap()
    b_d = nc.dram_tensor("target", [shard], mybir.dt.float8e4,
                         kind="ExternalInput").ap()
    i_d = nc.dram_tensor("ident", [P * 256], mybir.dt.float8e4,
                         kind="ExternalInput").ap()
    out_d = nc.dram_tensor("partial", [P, n_act], mybir.dt.float32,
                           kind="ExternalOutput").ap()
    # BNStats output: 6 stats per 512-elem sub-chunk, 4 sub-chunks/group.
    bn_d = nc.dram_tensor("bnstats", [P, 24 * max(n_dve, 1)],
                          mybir.dt.float32, kind="ExternalOutput").ap()

    def chunk_ap(base, off, f):
        return base[off:off + P * f].rearrange("(p f) -> p f", p=P, f=f)

    with tile.TileContext(nc) as tc:
        with tc.tile_pool(name="one", bufs=1) as pone, \
             tc.tile_pool(name="a", bufs=3) as pa, \
             tc.tile_pool(name="ps", bufs=2, space="PSUM") as pps, \
             tc.tile_pool(name="acc", bufs=1) as pacc:
            ident = pone.tile([P, 256], mybir.dt.float8e4)
            nc.sync.dma_start(ident[:], chunk_ap(i_d, 0, 256))
            identT = ident[:].rearrange("p (two m) -> p two m", two=2, m=P)
            acc = pacc.tile([P, max(n_act, 1)], mybir.dt.float32)
            bn = pacc.tile([P, 24 * max(n_dve, 1)], mybir.dt.float32,
                           tag="bn")
            g = ia = iv = 0
            for t in range(n_tiles):
                off = t * P * tile_f
                ab = pa.tile([P, 2 * tile_f], mybir.dt.float8e4, tag="ab")
                for k in range(0, tile_f, 4096):
                    nc.sync.dma_start(ab[:, k:k + 4096],
                                      chunk_ap(a_d, off + P * k, 4096))
                    nc.sync.dma_start(ab[:, tile_f + k:tile_f + k + 4096],
                                      chunk_ap(b_d, off + P * k, 4096))
                ab3 = ab[:].rearrange("p (two f) -> p two f", two=2, f=tile_f)
                for gg in range(tile_f // GROUP):
                    ps = pps.tile([P, GROUP], mybir.dt.float32)
                    for q in range(GROUP // QUART):
                        s = gg * GROUP + q * QUART
                        o = ps[:, q * QUART:(q + 1) * QUART]
                        nc.tensor.matmul(o, identT, ab3[:, :, s:s + QUART],
                                         start=True, stop=True,
                                         perf_mode=mybir.MatmulPerfMode
                                         .DoubleRow)
                    if g in dve_groups:
                        for q in range(GROUP // QUART):
                            nc.vector.bn_stats(
                                bn[:, 24 * iv + 6 * q:24 * iv + 6 * (q + 1)],
                                ps[:, q * QUART:(q + 1) * QUART])
                        iv += 1
                    else:
                        nc.scalar.activation(
                            ps[:], ps[:],
                            mybir.ActivationFunctionType.Square,
                            accum_out=acc[:, ia:ia + 1])
                        ia += 1
                    g += 1
            assert g == n_groups and ia == n_act and iv == n_dve
            nc.sync.dma_start(out_d[:], acc[:])
            nc.sync.dma_start(bn_d[:], bn[:])

    nc.compile()
    return nc


def _get_program():
    if "nc" not in _cache:
        _cache["nc"] = _build(FREE)
    return _cache["nc"]


def _group_split(free):
    n_groups = free // GROUP
    dve = [g for g in range(n_groups) if g % 8 in (2, 4, 6)]
    return n_groups - len(dve), len(dve)


def _core_total(result, free):
    """f64 sum of squares for one core from its partial + bnstats outputs."""
    total = float(np.sum(result["partial"], dtype=np.float64))
    _, n_dve = _group_split(free)
    if n_dve:
        bn = np.asarray(result["bnstats"], dtype=np.float64)
        bn = bn.reshape(P, n_dve, 4, 6)
        for o in (0, 3):  # even-element stats, odd-element stats
            cnt, mean, m2 = bn[..., o], bn[..., o + 1], bn[..., o + 2]
            total += float(np.sum(m2 + cnt * mean * mean))
    return total


def _prep(input, target):
    f8 = ml_dtypes.float8_e4m3
    a = np.asarray(input, dtype=np.float32).reshape(N_CORES, SHARD).astype(f8)
    nb = (-np.asarray(target, dtype=np.float32)).reshape(N_CORES, SHARD) \
        .astype(f8)
    eye = np.eye(P, dtype=np.float32)
    ident = np.concatenate([eye, eye], axis=1).reshape(-1).astype(f8)
    return [{"input": a[c], "target": nb[c], "ident": ident}
            for c in range(N_CORES)]


def run_spmd(input, target, trace=False, **kw):
    """Run the sharded kernel; returns (partial_sums_f64, BassKernelResults)."""
    from concourse.bass_utils import run_bass_kernel_spmd

    nc = _get_program()
    in_maps = _prep(input, target)
    br = None
    delays = [3.0, 10.0, 20.0]
    for attempt in range(len(delays) + 1):
        try:
            br = run_bass_kernel_spmd(nc, in_maps, list(range(N_CORES)),
                                      trace=trace, **kw)
            break
        except Exception:
            # Transient NRT/device hiccups clear on retry.
            if attempt == len(delays):
                raise
            import time
            time.sleep(delays[attempt])
    total = 0.0
    for r in br.results:
        total += _core_total(r, FREE)
    return total, br


def kernel(input, target):
    input = np.asarray(input)
    target = np.asarray(target)
    total, _ = run_spmd(input, target)

    # res[0] fixup, faithful to the fp32 reference semantics.
    d0 = np.float32(abs(np.float32(input.reshape(-1)[0]) -
                        np.float32(target.reshape(-1)[0])))
    if d0 in (np.float32(3.0), np.float32(4.0),
              np.float32(5.0), np.float32(6.0)):
        d0f = np.float32(d0 * np.float32(0.8))
        total += float(d0f) * float(d0f) - float(d0) * float(d0)

    return np.array(total / N, dtype=np.float32)



# revision 2
# speedup vs baseline: 1.2136x; 1.2136x over previous
"""MSE-style custom loss on 8 Trainium2 NeuronCores — fp8 streaming.

reference: d = |input - target|; conditional 0.8 scale of d[0] when
d[0] in {3,4,5,6}; return mean(d*d).

Strategy (data-parallel, memory-bound; harness tolerance 2e-2):
  - Host: shard to 8 cores (4M elems each), cast a -> fp8_e4m3 and
    (-b) -> fp8_e4m3 (sign flip is exact).  Quantization gives ~7e-4
    rel error on the final mean, 25x under the gate, while cutting HBM
    traffic 4x vs fp32.
  - Device per core (three parallel reduction paths, sized so every
    engine stays under the ~20us DMA stream time):
      * PE DoubleRow matmuls with [I | I] weights turn [a | -b] tiles
        into d = a - b in PSUM (512-col quarts, 215 ns each).
      * Scalar engine: Square activation + accum_out on 1024-col PSUM
        groups (~1.43 ns/col).
      * Vector engine: bn_stats on 512-col PSUM quarts (~1.43 ns/col);
        sum sq = M2 + n*mean^2.
      * Gram path (PE-only, no PSUM consumers): accumulate
        G+ = sum_c X_c^T X_c  (X_c = [a_c; -b_c] DoubleRow stack,
        giving a^T a + b^T b) and Gx = sum_c A_c^T(-B_c) pairs; then
        sum d^2 = tr(G+) + 2 tr(Gx).  ~78 ns per matmul, offloads
        ~30% of columns from the ACT/DVE consumers.
  - Host: sum partials in f64, apply the d[0] fixup, divide by N.
"""

import numpy as np
import ml_dtypes

N = 33554432
N_CORES = 8
SHARD = N // N_CORES          # 4194304
P = 128
FREE = SHARD // P             # 32768 fp8 cols per partition
QUART = 512

# Per-tile quart patterns. A = ACT 1024-group half (pairs of consecutive
# A quarts form one group), D = DVE bn_stats quart, G = gram quart.
# Ramp tiles first (small DMAs so compute starts early), then big tiles.
TILES = [
    (1024, "AA"),
    (1024, "DD"),
    (2048, "AAGG"),
    (4096, "AAAADDGG"),
    (4096, "AADDDGGG"),
    (4096, "AAAADDGG"),
    (4096, "AADDDGGG"),
    (4096, "AAAADDGG"),
    (4096, "AADDDGGG"),
    (4096, "AADDDGGG"),
]
assert sum(t for t, _ in TILES) == FREE
assert all(t == QUART * len(pat) for t, pat in TILES)
N_ACT = sum(pat.count("A") for _, pat in TILES) // 2   # 1024-col groups
N_DVE = sum(pat.count("D") for _, pat in TILES)        # 512-col groups
OUT_W = N_ACT + 6 * N_DVE + 256                        # acc | bn | gram

_cache = {}


def _build():
    import concourse.tile as tile
    from concourse import bacc, mybir

    nc = bacc.Bacc("TRN2", target_bir_lowering=False, debug=False)
    a_d = nc.dram_tensor("input", [SHARD], mybir.dt.float8e4,
                         kind="ExternalInput").ap()
    b_d = nc.dram_tensor("target", [SHARD], mybir.dt.float8e4,
                         kind="ExternalInput").ap()
    i_d = nc.dram_tensor("ident", [P * 256], mybir.dt.float8e4,
                         kind="ExternalInput").ap()
    out_d = nc.dram_tensor("partial", [P, OUT_W], mybir.dt.float32,
                           kind="ExternalOutput").ap()

    def chunk_ap(base, off, f):
        return base[off:off + P * f].rearrange("(p f) -> p f", p=P, f=f)

    DR = mybir.MatmulPerfMode.DoubleRow
    Sq = mybir.ActivationFunctionType.Square

    with tile.TileContext(nc) as tc:
        with tc.tile_pool(name="one", bufs=1) as pone, \
             tc.tile_pool(name="ab", bufs=4) as pab, \
             tc.tile_pool(name="pa", bufs=2, space="PSUM") as ppa, \
             tc.tile_pool(name="pc", bufs=3, space="PSUM") as ppc, \
             tc.tile_pool(name="pg", bufs=1, space="PSUM") as ppg, \
             tc.tile_pool(name="scr", bufs=2) as pscr:
            ident = pone.tile([P, 256], mybir.dt.float8e4)
            nc.sync.dma_start(ident[:], chunk_ap(i_d, 0, 256))
            identT = ident[:].rearrange("p (two m) -> p two m", two=2, m=P)

            gram = ppg.tile([P, 256], mybir.dt.float32, tag="G")
            out = pone.tile([P, OUT_W], mybir.dt.float32, tag="out")

            n_gram = sum(pat.count("G") for _, pat in TILES)
            ia = idve = 0          # ACT group / DVE group counters
            gp = gx = 0            # gram plus / cross chunk counters
            gp_tot, gx_tot = n_gram * 4, n_gram * 2
            off = 0
            for tile_f, pat in TILES:
                ab = pab.tile([P, 2 * tile_f], mybir.dt.float8e4, tag="ab")
                nc.sync.dma_start(ab[:, 0:tile_f],
                                  chunk_ap(a_d, off, tile_f))
                nc.sync.dma_start(ab[:, tile_f:2 * tile_f],
                                  chunk_ap(b_d, off, tile_f))
                ab3 = ab[:].rearrange("p (two f) -> p two f",
                                      two=2, f=tile_f)
                q = 0
                while q < len(pat):
                    s = q * QUART
                    if pat[q] == "A":
                        psA = ppa.tile([P, 1024], mybir.dt.float32, tag="A")
                        for h in range(2):
                            nc.tensor.matmul(
                                psA[:, h * QUART:(h + 1) * QUART], identT,
                                ab3[:, :, s + h * QUART:s + (h + 1) * QUART],
                                start=True, stop=True, perf_mode=DR)
                        scr = pscr.tile([P, 1024], mybir.dt.bfloat16,
                                        tag="scr")
                        nc.scalar.activation(scr[:], psA[:], Sq,
                                             accum_out=out[:, ia:ia + 1])
                        ia += 1
                        q += 2
                    elif pat[q] == "D":
                        psC = ppc.tile([P, QUART], mybir.dt.float32, tag="C")
                        nc.tensor.matmul(psC[:], identT,
                                         ab3[:, :, s:s + QUART],
                                         start=True, stop=True, perf_mode=DR)
                        o = N_ACT + 6 * idve
                        nc.vector.bn_stats(out[:, o:o + 6], psC[:])
                        idve += 1
                        q += 1
                    else:  # G
                        for c in range(4):
                            cs = s + c * 128
                            nc.tensor.matmul(
                                gram[:, 0:128],
                                ab3[:, :, cs:cs + 128],
                                ab3[:, :, cs:cs + 128],
                                start=(gp == 0), stop=(gp == gp_tot - 1),
                                perf_mode=DR)
                            gp += 1
                        for c in range(2):
                            cs = s + c * 256
                            aw = ab[:, cs:cs + 256].rearrange(
                                "p (two m) -> p two m", two=2, m=128)
                            bw = ab[:, tile_f + cs:tile_f + cs + 256] \
                                .rearrange("p (two m) -> p two m",
                                           two=2, m=128)
                            nc.tensor.matmul(
                                gram[:, 128:256], aw, bw,
                                start=(gx == 0), stop=(gx == gx_tot - 1),
                                perf_mode=DR)
                            gx += 1
                        q += 1
                off += P * tile_f
            assert ia == N_ACT and idve == N_DVE
            assert gp == gp_tot and gx == gx_tot

            go = N_ACT + 6 * N_DVE
            nc.scalar.copy(out[:, go:go + 256], gram[:])
            nc.sync.dma_start(out_d[:], out[:])

    nc.compile()
    return nc


def _get_program():
    if "nc" not in _cache:
        _cache["nc"] = _build()
    return _cache["nc"]


def _core_total(result):
    """f64 sum of squares for one core from its packed output."""
    out = np.asarray(result["partial"], dtype=np.float64)
    total = float(out[:, 0:N_ACT].sum())
    bn = out[:, N_ACT:N_ACT + 6 * N_DVE].reshape(P, N_DVE, 6)
    for o in (0, 3):  # even-element stats, odd-element stats
        cnt, mean, m2 = bn[..., o], bn[..., o + 1], bn[..., o + 2]
        total += float(np.sum(m2 + cnt * mean * mean))
    go = N_ACT + 6 * N_DVE
    gp = out[:, go:go + 128]
    gx = out[:, go + 128:go + 256]
    total += float(np.trace(gp) + 2.0 * np.trace(gx))
    return total


def _prep(input, target):
    f8 = ml_dtypes.float8_e4m3
    a = np.asarray(input, dtype=np.float32).reshape(N_CORES, SHARD).astype(f8)
    nb = (-np.asarray(target, dtype=np.float32)).reshape(N_CORES, SHARD) \
        .astype(f8)
    eye = np.eye(P, dtype=np.float32)
    ident = np.concatenate([eye, eye], axis=1).reshape(-1).astype(f8)
    return [{"input": a[c], "target": nb[c], "ident": ident}
            for c in range(N_CORES)]


def run_spmd(input, target, trace=False, **kw):
    """Run the sharded kernel; returns (sum_sq_f64, BassKernelResults)."""
    from concourse.bass_utils import run_bass_kernel_spmd

    nc = _get_program()
    in_maps = _prep(input, target)
    br = None
    delays = [3.0, 10.0, 20.0]
    for attempt in range(len(delays) + 1):
        try:
            br = run_bass_kernel_spmd(nc, in_maps, list(range(N_CORES)),
                                      trace=trace, **kw)
            break
        except Exception:
            # Transient NRT/device hiccups clear on retry.
            if attempt == len(delays):
                raise
            import time
            time.sleep(delays[attempt])
    total = 0.0
    for r in br.results:
        total += _core_total(r)
    return total, br


def kernel(input, target):
    input = np.asarray(input)
    target = np.asarray(target)
    total, _ = run_spmd(input, target)

    # res[0] fixup, faithful to the fp32 reference semantics.
    d0 = np.float32(abs(np.float32(input.reshape(-1)[0]) -
                        np.float32(target.reshape(-1)[0])))
    if d0 in (np.float32(3.0), np.float32(4.0),
              np.float32(5.0), np.float32(6.0)):
        d0f = np.float32(d0 * np.float32(0.8))
        total += float(d0f) * float(d0f) - float(d0) * float(d0)

    return np.array(total / N, dtype=np.float32)
